# revision 42
# baseline (speedup 1.0000x reference)
"""CRF loss (mean(log_partition - path_score)) on 8 Trainium2 cores.

Per core (data-parallel over batch, 128 rows/core): rescaled forward
algorithm in probability space, state-major layout [states on
partitions, batch rows on free], so TensorE contracts the state dim:

    q_s = E^T (F_s * q_{s-1}),  E = exp(trans),  F_s = exp(e_s - C0)

The numerator (forced-path score) rides along as a second 48-state
block stacked on partitions (den rows 0:48, num rows 64:112) with a
block-diagonal E2; its factors are exp(enum_s) where enum is the
HOST-masked emissions (e where k == tag, else -1e30), so one ACT exp
produces onehot(tag)*exp(e) with no DVE work.

Performance structure (458us baseline -> ~308us):
- Factors are built batch-major per 64-step chunk (ACT exp only) and
  moved to state-major by ONE chunked DMA xbar transpose with a 3D
  output [128, 64, 128] (per-step block transposes). PE runs nothing
  but the chain matmuls.
- All DMA (loads + transposes) stays on the SP HWDGE ring; issuing any
  of it from the ACT ring produced nondeterministic corruption.
- _strip_redundant_ldweights removes the per-matmul InstLdweights
  emitted by the bacc pipeline when consecutive matmuls share E2
  (weights persist in the PE array; ~150ns x 1500 saved).
- The batch is split into three streams (43/43/42 rows) interleaved
  per step: while PE runs one stream's matmul, DVE runs another's
  multiply, hiding the serial scan's DVE<->PE handoff latency and
  keeping both engines ~90% busy (engine pipes go cold when idle,
  +80ns on the first op after a gap).
- Den chain rescale is a constant exp(-C0) per step folded into the
  factors (C0 = 4.87 = measured drift; residual +-12 over 511 steps).
  Num chain rescales per-row at s = 128/256/384 (drift +-60 between);
  the scale is computed off the critical path and applied one step
  late (a per-column scale commutes through the matmul). ln(m) uses
  2*ln(sqrt(m)) because the ACT Ln table only covers ln(x) in +-44,
  and all Ln/Sqrt evaluation is deferred to the readout and batched
  per function (each ACT function switch costs a 1.3us table load).
- The init state q0 is built batch-major and xbar-transposed into
  SBUF, so stream slices are free-dim (PE APs require partition bases
  in {0,32,64} which would force unequal streams).
"""

import numpy as np

B, T, K = 1024, 512, 48
NCORES = 8
BPC = B // NCORES          # 128 batch rows per core
HB = BPC // 2              # (legacy) half width
# three batch streams (base, width); bases must be PE-quadrant aligned
STREAMS = ((0, 43), (43, 43), (86, 42))
NS = len(STREAMS)
KS = 112                   # stacked partition dim: den [0:48], num [64:112]
NUM0 = 64                  # num-chain partition base
GW = 128                   # G free width per step (den 0:48, num 64:112, pad 0)
C0 = 4.87                  # constant per-step den rescale, folded into exp(e-C0)
TC = 64                    # timesteps per pipeline chunk
NCHUNK = T // TC
NSUB = 8                   # sub-ops per chunk
RESC = (128, 256, 384)     # num-chain rescale steps

_CACHE = {}


def _build_program():
    from contextlib import ExitStack

    import concourse.bacc as bacc
    import concourse.bass as bass
    import concourse.tile as tile
    from concourse import mybir
    from concourse.masks import make_identity

    f32 = mybir.dt.float32
    bf16 = mybir.dt.bfloat16
    Exp = mybir.ActivationFunctionType.Exp
    Ln = mybir.ActivationFunctionType.Ln
    Sqrt = mybir.ActivationFunctionType.Sqrt
    AX = mybir.AxisListType.X
    EQ = mybir.AluOpType.is_equal
    MUL = mybir.AluOpType.mult

    nc = bacc.Bacc("TRN2", target_bir_lowering=False, debug=False)

    e_d = nc.dram_tensor("e", [BPC, T, K], bf16, kind="ExternalInput")
    tags_d = nc.dram_tensor("tags", [BPC, T], f32, kind="ExternalInput")
    enum_d = nc.dram_tensor("enum", [BPC, T, K], bf16, kind="ExternalInput")
    trans_d = nc.dram_tensor("trans", [K, K], f32, kind="ExternalInput")
    start_d = nc.dram_tensor("start", [K], f32, kind="ExternalInput")
    end_d = nc.dram_tensor("end", [K], f32, kind="ExternalInput")
    out_d = nc.dram_tensor("out", [1, 1], f32, kind="ExternalOutput")

    with tile.TileContext(nc) as tc, ExitStack() as ctx:
        const = ctx.enter_context(tc.tile_pool(name="const", bufs=1))
        epool = ctx.enter_context(tc.tile_pool(name="epool", bufs=3))
        gpool = ctx.enter_context(tc.tile_pool(name="gpool", bufs=3))
        ftpool = ctx.enter_context(tc.tile_pool(name="ftpool", bufs=3))
        upool = ctx.enter_context(tc.tile_pool(name="upool", bufs=6))
        small = ctx.enter_context(tc.tile_pool(name="small", bufs=4))
        lnpool = ctx.enter_context(tc.tile_pool(name="lnpool", bufs=1))
        qps = [ctx.enter_context(tc.tile_pool(name=f"qps{i}", bufs=2, space="PSUM"))
               for i in range(NS)]
        mps = ctx.enter_context(tc.tile_pool(name="mps", bufs=1, space="PSUM"))

        def bcast(ap, shape_ap):
            return bass.AP(tensor=ap.tensor, offset=ap.offset, ap=shape_ap)

        # ---------------- constants ----------------
        ident = const.tile([128, 128], bf16)
        make_identity(nc, ident[:])

        # Block-diagonal exp(trans) [112, 112] bf16
        trans2 = const.tile([KS, KS], f32)
        nc.vector.memset(trans2[:], -1e30)
        nc.sync.dma_start(out=trans2[0:K, 0:K], in_=trans_d.ap())
        nc.sync.dma_start(out=trans2[NUM0:NUM0 + K, NUM0:NUM0 + K], in_=trans_d.ap())
        E2 = const.tile([KS, KS], bf16)
        nc.scalar.activation(E2[:], trans2[:], Exp)

        # start replicated across partitions [128, 48] f32 (needed by init;
        # first in the gpsimd queue so a0 is unblocked early)
        start_rep = const.tile([BPC, K], f32)
        nc.gpsimd.dma_start(
            out=start_rep[:], in_=bcast(start_d.ap(), [[0, BPC], [1, K]])
        )

        # tiny first-step emission column for the init (its own DMA so the
        # init does not wait for the full chunk-0 load)
        e0col = const.tile([BPC, 1, K], bf16)
        nc.sync.dma_start(out=e0col[:], in_=e_d.ap()[:, 0:1, :])

        # ones [1, 48] bf16 (num-rescale broadcast matmul) and
        # ones [KS, 1] bf16 (num-rescale partition-sum matmul)
        ones48 = const.tile([1, K], bf16)
        nc.vector.memset(ones48[:], 1.0)
        onesKS = const.tile([KS, 1], bf16)
        nc.vector.memset(onesKS[:], 1.0)

        # per-partition bias constant -C0 for the bulk exp
        negc0 = const.tile([128, 1], f32)
        nc.vector.memset(negc0[:], -C0)

        # ---------------- factor pipeline ----------------
        # G chunk layout [128, TC, GW] bf16:
        #   G[:, i, 0:48]    = exp(e[:, 64c+i, :] - C0)             (den)
        #   G[:, i, 64:112]  = exp(enum[:, 64c+i-1, :])             (num)
        # where enum = e masked to -1e30 off the tag path (host-built), so
        # exp(enum) IS onehot(tag)*exp(e). All factor math lives on ACT;
        # DVE runs nothing but the latency-critical chain multiplies.
        # gap columns 48:64 stay zero (fT rows 48:64 must not be NaN).
        gtiles = [None] * NCHUNK
        etiles = [None] * NCHUNK
        entiles = [None] * NCHUNK
        ftcs = [None] * NCHUNK

        def emit_chunk(c):
            ech = epool.tile([BPC, TC, K], bf16, tag="ech")
            if c == 0:
                # split chunk 0's loads: the first 16 steps' slice lands
                # ~4us sooner, unblocking the first factor exps + transpose
                nc.sync.dma_start(out=ech[:, 0:16, :], in_=e_d.ap()[:, 0:16, :])
                nc.sync.dma_start(
                    out=ech[:, 16:TC, :], in_=e_d.ap()[:, 16:TC, :]
                )
            else:
                nc.sync.dma_start(
                    out=ech[:], in_=e_d.ap()[:, c * TC:(c + 1) * TC, :]
                )
            # ench holds enum shifted one step back (enum[64c-1+i]), so the
            # num factor for step i never reads across chunk tiles (a sliver
            # cross-chunk ACT op raced the chunked xbar transpose).
            ench = epool.tile([BPC, TC, K], bf16, tag="ench")
            if c == 0:
                nc.sync.dma_start(
                    out=ench[:, 1:16, :], in_=enum_d.ap()[:, 0:15, :]
                )
                nc.sync.dma_start(
                    out=ench[:, 16:TC, :], in_=enum_d.ap()[:, 15:TC - 1, :]
                )
            else:
                nc.sync.dma_start(
                    out=ench[:],
                    in_=enum_d.ap()[:, c * TC - 1:(c + 1) * TC - 1, :],
                )
            G = gpool.tile([BPC, TC, GW], bf16, tag="G")
            etiles[c] = ech
            entiles[c] = ench
            gtiles[c] = G
            nc.gpsimd.memset(G[:, :, K:NUM0], 0.0)
            sub = TC // NSUB
            for j in range(NSUB):
                i0, i1 = j * sub, (j + 1) * sub
                # den factors (ACT)
                nc.scalar.activation(
                    G[:, i0:i1, 0:K], ech[:, i0:i1, :], Exp, bias=negc0[:BPC, 0:1]
                )
                # num factors (ACT; skip the unused i=0 slot of chunk 0)
                k0 = 1 if (c == 0 and i0 == 0) else i0
                nc.scalar.activation(
                    G[:, k0:i1, NUM0:NUM0 + K], ench[:, k0:i1, :], Exp
                )

        def issue_transposes(c, pieces=1):
            # One xbar transpose for the whole chunk: out[p, i, b] = G[b, i, p]
            # (3D out folds the extra dim into the partition dim). Alternate
            # the two HWDGE rings (SP / ACT) across chunks. Chunk 0 is split
            # into pieces so the chain can start as soon as the first steps'
            # factors are transposed (startup latency).
            ftc = ftpool.tile([GW, TC, BPC], bf16, tag="fT")
            eng = nc.sync
            step = TC // pieces
            for p in range(pieces):
                eng.dma_start_transpose(
                    out=ftc[:, p * step:(p + 1) * step, :],
                    in_=gtiles[c][:, p * step:(p + 1) * step, :],
                )
            ftcs[c] = ftc

        # ---------------- init (t = 0) ----------------
        # q0 batch-major [128, 112] bf16: cols 0:48 = exp(e0 + start),
        # cols 48:64 = 0, cols 64:112 = exp(start) (num-mask shifts by one,
        # so the t=0 num state is exp(start) un-masked).
        a0 = small.tile([BPC, K], f32, tag="a0")
        nc.vector.tensor_add(a0[:], e0col[:, 0, :], start_rep[:])
        q0bm = small.tile([BPC, GW], bf16, tag="q0bm")
        nc.vector.memset(q0bm[:, K:NUM0], 0.0)
        nc.scalar.activation(q0bm[:, 0:K], a0[:], Exp)
        nc.scalar.activation(q0bm[:, NUM0:NUM0 + K], start_rep[:], Exp)
        # state-major init state via the xbar (SBUF dest, so stream slices
        # are free-dim and need no partition-base alignment); cols 112:128
        # transpose into rows 112:128 which the chain never reads. The DMA
        # itself is issued after the chunk-0 loads/transposes so it does not
        # head-of-line block them on the SP ring behind the init ACT ops.
        q0sm = const.tile([GW, BPC], bf16)
        qprev = [q0sm[0:KS, b0:b0 + w] for (b0, w) in STREAMS]

        # SP-ring order: chunk-0 loads, init transpose, then chunk-0's
        # factor transposes IMMEDIATELY (chunk 1/2 loads are only needed at
        # s=64+ and would otherwise delay the chain start by ~10us).
        emit_chunk(0)
        nc.sync.dma_start_transpose(out=q0sm[:], in_=q0bm[:])
        issue_transposes(0, pieces=8)
        emit_chunk(1)
        emit_chunk(2)
        issue_transposes(1)

        # readout-only constants (emitted after the hot prologue; they only
        # need to be ready by the readout)
        ends = const.tile([KS, 2], f32)
        nc.vector.memset(ends[:], -1e30)
        end_col = end_d.ap().rearrange("(k one) -> k one", one=1)
        nc.gpsimd.dma_start(out=ends[0:K, 0:1], in_=end_col)
        nc.gpsimd.dma_start(out=ends[NUM0:NUM0 + K, 1:2], in_=end_col)
        lhsT_end = const.tile([KS, 2], bf16)
        nc.scalar.activation(lhsT_end[:], ends[:], Exp)
        iota48 = const.tile([BPC, K], bf16)
        nc.gpsimd.iota(
            iota48[:], pattern=[[1, K]], base=0, channel_multiplier=0,
            allow_small_or_imprecise_dtypes=True,
        )
        tags_f = const.tile([BPC, 1], f32)
        nc.sync.dma_start(out=tags_f[:], in_=tags_d.ap()[:, T - 1:T])
        tags_bf = const.tile([BPC, 1], bf16)
        nc.scalar.copy(tags_bf[:], tags_f[:])

        lnms = []  # (stream, [1, w] f32 stashed rescale m tiles)
        pending = [None] * NS  # deferred per-stream rescale tiles

        # ---------------- main chain ----------------
        for s in range(1, T):
            c, srel = divmod(s, TC)
            if srel == 0 and c + 1 < NCHUNK:
                if c + 2 < NCHUNK:
                    emit_chunk(c + 2)
                issue_transposes(c + 1)
            ft = ftcs[c]
            resc = s in RESC
            us = [None] * NS
            for h, (b0, w) in enumerate(STREAMS):
                u = upool.tile([KS, w], bf16, tag=f"u{h}")
                nc.vector.tensor_mul(
                    u[:], qprev[h], ft[0:KS, srel, b0:b0 + w]
                )
                if pending[h] is not None:
                    # apply the scale computed at the previous step (a scale
                    # per batch column commutes through the E2 contraction)
                    rr = pending[h]
                    pending[h] = None
                    nc.vector.tensor_mul(
                        u[NUM0:NUM0 + K, :], u[NUM0:NUM0 + K, :],
                        rr[NUM0:NUM0 + K, :],
                    )
                us[h] = u
                qn = qps[h].tile([KS, w], f32, tag="q")
                mi = nc.tensor.matmul(
                    qn[:], lhsT=E2[:], rhs=u[:], start=True, stop=True
                )
                # E2 stays resident in the PE array: re-load only on the
                # first step and after rescale matmuls clobbered it.
                if s > 1 and (s - 1) not in RESC:
                    mi.ins.ldweights = False
                qprev[h] = qn[:]
            if resc:
                # Compute the num-chain rescale OFF the critical path: the
                # scale lands on u at step s+1, so the m/recip/broadcast
                # pipeline overlaps the running chain.
                for h, (b0, w) in enumerate(STREAMS):
                    u = us[h]
                    # m[b] = sum_i u_num[i,b] (the single nonzero = row max)
                    m = mps.tile([1, w], f32, tag="mm")
                    nc.tensor.matmul(
                        m[:], lhsT=onesKS[NUM0:NUM0 + K, 0:1],
                        rhs=u[NUM0:NUM0 + K, :], start=True, stop=True,
                    )
                    # No ACT here: a function switch (Exp->Sqrt->Ln) costs a
                    # 1.3us ACT_TABLE_LOAD each way mid-chain. Stash m and
                    # take logs at the readout, batched per function.
                    # stash + cast on the (idle-at-event) ACT engine; DVE
                    # is the saturated engine during rescale events
                    mcp = lnpool.tile([1, w], f32, tag=f"m{h}_{s}")
                    nc.scalar.copy(mcp[:], m[:])
                    lnms.append((h, w, mcp))
                    recf = small.tile([1, w], f32, tag=f"recf{h}")
                    nc.vector.reciprocal(recf[:], m[:])
                    recb = small.tile([1, w], bf16, tag=f"rec{h}")
                    nc.scalar.copy(recb[:], recf[:])
                    rr = mps.tile([KS, w], f32, tag="rr")
                    nc.tensor.matmul(
                        rr[NUM0:NUM0 + K, :], lhsT=ones48[:], rhs=recb[:],
                        start=True, stop=True,
                    )
                    pending[h] = rr

        # ---------------- readout ----------------
        # Prefetch the Sqrt ACT table during the chain tail: gate it on a
        # value written at the last step so the scheduler cannot hoist it
        # before the factor Exps (their table switch would undo it).
        marker = small.tile([1, 1], f32, tag="marker")
        nc.vector.tensor_copy(marker[:], qprev[0][0:1, 0:1])
        sqpre = small.tile([1, 1], f32, tag="sqpre")
        nc.scalar.activation(sqpre[:], marker[:], Sqrt)
        oh511 = small.tile([BPC, K], bf16, tag="oh511")
        t511 = tags_bf[:, 0:1]
        nc.vector.tensor_tensor(
            out=oh511[:],
            in0=bcast(t511, [t511.ap[0], [0, K]]),
            in1=iota48[:],
            op=EQ,
        )
        ohT_ps = mps.tile([KS, BPC], bf16, tag="rr")
        nc.tensor.transpose(
            out=ohT_ps[NUM0:NUM0 + K, :], in_=oh511[:], identity=ident[:]
        )
        maskF = const.tile([KS, BPC], bf16)
        nc.vector.memset(maskF[:], 0.0)
        nc.vector.memset(maskF[0:K, :], 1.0)
        nc.scalar.copy(maskF[NUM0:NUM0 + K, :], ohT_ps[NUM0:NUM0 + K, :])

        # Readout matmuls first, then ALL sqrts, then ALL lns — one ACT
        # table load per function instead of one per op. r0/r1 live in the
        # (now-idle) per-stream chain PSUM pools, so nothing serializes on
        # the shared mps banks.
        r0s, r1s = [], []
        for h, (b0, w) in enumerate(STREAMS):
            u_f = upool.tile([KS, w], bf16, tag=f"u{h}")
            nc.vector.tensor_mul(
                u_f[:], qprev[h], maskF[:, b0:b0 + w]
            )
            r0 = qps[h].tile([1, w], f32, tag="q")
            nc.tensor.matmul(
                r0[:], lhsT=lhsT_end[:, 0:1], rhs=u_f[:], start=True, stop=True
            )
            r1 = qps[h].tile([1, w], f32, tag="q")
            nc.tensor.matmul(
                r1[:], lhsT=lhsT_end[:, 1:2], rhs=u_f[:], start=True, stop=True
            )
            r0s.append(r0)
            r1s.append(r1)

        # sqrt batch: r1 (num readout) and every stashed rescale m can sit
        # at e^+-60, past the ACT Ln table range — eval ln(x) = 2 ln(sqrt x)
        sq_r1, sq_m = [], []
        for h, (b0, w) in enumerate(STREAMS):
            s_ = small.tile([1, w], f32, tag=f"r1sq{h}")
            nc.scalar.activation(s_[:], r1s[h][:], Sqrt)
            sq_r1.append(s_)
        for idx, (hh, w_, mcp) in enumerate(lnms):
            s_ = lnpool.tile([1, w_], f32, tag=f"msq{idx}")
            nc.scalar.activation(s_[:], mcp[:], Sqrt)
            sq_m.append((hh, w_, s_))
        # ln batch
        L0s, L1hs = [], []
        for h, (b0, w) in enumerate(STREAMS):
            L0 = small.tile([1, w], f32, tag=f"L0{h}")
            nc.scalar.activation(L0[:], r0s[h][:], Ln)
            L0s.append(L0)
            L1h = small.tile([1, w], f32, tag=f"L1h{h}")
            nc.scalar.activation(L1h[:], sq_r1[h][:], Ln)
            L1hs.append(L1h)
        lnm_half = []
        for idx, (hh, w_, s_) in enumerate(sq_m):
            lm = lnpool.tile([1, w_], f32, tag=f"lnm{idx}")
            nc.scalar.activation(lm[:], s_[:], Ln)
            lnm_half.append((hh, lm))

        # g = L0 - 2*(L1h + sum lnm_half)
        g2 = small.tile([1, BPC], f32, tag="g2")
        for h, (b0, w) in enumerate(STREAMS):
            acc = small.tile([1, w], f32, tag=f"acc{h}")
            nc.vector.tensor_copy(acc[:], L1hs[h][:])
            for hh, lm in lnm_half:
                if hh == h:
                    nc.vector.tensor_add(acc[:], acc[:], lm[:])
            acc2 = small.tile([1, w], f32, tag=f"acc2{h}")
            nc.vector.tensor_scalar_mul(acc2[:], acc[:], 2.0)
            nc.vector.tensor_tensor(
                g2[0:1, b0:b0 + w], L0s[h][:], acc2[:],
                op=mybir.AluOpType.subtract,
            )

        total = small.tile([1, 1], f32, tag="total")
        nc.vector.reduce_sum(total[:], g2[:], axis=AX)
        nc.sync.dma_start(out=out_d.ap(), in_=total[:])

    nc.compile()
    import os
    if not os.environ.get("NO_LDW_STRIP"):
        _strip_redundant_ldweights(nc, mybir)
    return nc


def _strip_redundant_ldweights(nc, mybir):
    """Drop InstLdweights that reload the exact weights already resident.

    The bacc pipeline pairs every InstMatmult with its own InstLdweights even
    when consecutive matmuls share the stationary tensor (E2 here, 1022x).
    Weights persist in the PE array between matmuls, so a reload of an
    identical physical AP is a no-op costing ~150ns. Only drop loads that
    carry no semaphore ops; reset tracking per block.
    """
    removed = 0
    for f in nc.m.functions:
        for blk in f.blocks:
            insts = blk.instructions
            new = []
            last_w = None
            for inst in insts:
                if isinstance(inst, mybir.InstLdweights):
                    wkey = (str(inst.ins[0]), str(inst.is_transpose),
                            str(inst.tile_position), str(inst.perf_mode))
                    si = inst.sync_info
                    plain = si is None or (not si.on_wait and not si.on_update)
                    if plain and wkey == last_w:
                        removed += 1
                        continue
                    last_w = wkey
                new.append(inst)
            if removed:
                blk.instructions = new
    return removed


def _get_program():
    if "nc" not in _CACHE:
        _CACHE["nc"] = _build_program()
    return _CACHE["nc"]


def kernel(**inputs):
    from concourse.bass_utils import run_bass_kernel_spmd

    e = np.asarray(inputs["emissions"], np.float32)
    tags = np.asarray(inputs["tags"])
    mask = np.asarray(inputs["mask"], np.float32)
    trans = np.asarray(inputs["transitions"], np.float32)
    start = np.asarray(inputs["start_transitions"], np.float32)
    end = np.asarray(inputs["end_transitions"], np.float32)
    assert np.all(mask == 1.0), "kernel specialized for mask == ones"

    nc = _get_program()
    import ml_dtypes
    onehot = tags[..., None] == np.arange(K, dtype=tags.dtype)
    enum = np.where(onehot, e, np.float32(-1e30)).astype(ml_dtypes.bfloat16)
    e_bf = e.astype(ml_dtypes.bfloat16)
    in_maps = []
    for ci in range(NCORES):
        sl = slice(ci * BPC, (ci + 1) * BPC)
        in_maps.append({
            "e": np.ascontiguousarray(e_bf[sl]),
            "enum": np.ascontiguousarray(enum[sl]),
            "tags": np.ascontiguousarray(tags[sl]).astype(np.float32),
            "trans": trans,
            "start": start,
            "end": end,
        })
    res = run_bass_kernel_spmd(nc, in_maps, list(range(NCORES)))
    tot = sum(float(res.results[ci]["out"][0, 0]) for ci in range(NCORES))
    return np.asarray(tot / B + (T - 1) * C0, dtype=np.float32)
